# revision 12
# baseline (speedup 1.0000x reference)
"""BiLSTM-CRF kernel for 8 Trainium2 NeuronCores.

Data-parallel: batch (32) sharded 4-per-core across 8 cores. The
device kernel computes the FLOP-dominant layer-0 BiLSTM input
projections (x @ Wih^T for both directions, 768->512 contraction per
token) as tiled PE matmuls; the strictly sequential time scans / CRF
recursion run vectorized on host on the projected activations.
"""
import sys
import numpy as np

sys.path.insert(0, "/opt/trn_rl_repo")

V, D, H, NLAYERS, K = 30522, 768, 64, 4, 9
B, T = 32, 256
NCORES = 8
BS = B // NCORES          # 4 sequences per core
TOK = BS * T              # 1024 tokens per core
KD = D                    # layer-0 contraction dim
NOUT = 8 * H              # fw(4H) + bw(4H) = 512 projection outputs


def _build_bass():
    import concourse.bass as bass
    import concourse.mybir as mybir

    f32 = mybir.dt.float32
    nc = bass.Bass(target_bir_lowering=False, debug=False)

    KC = KD // 128            # 6 contraction chunks
    TC = TOK // 128           # 8 token tiles

    # host pre-packs x as [128, KC*TOK] (k-chunk-major cols) and w as
    # [128, KC*NOUT] so each input is ONE contiguous DMA.
    xT = nc.declare_dram_parameter("xT", [128, KC * TOK], f32, isOutput=False)
    w = nc.declare_dram_parameter("w", [128, KC * NOUT], f32, isOutput=False)
    out = nc.declare_dram_parameter("out", [TOK, NOUT], f32, isOutput=True)

    with (
        nc.Block() as block,
        nc.semaphore("dma_sem") as dma_sem,
        nc.semaphore("mm_sem") as mm_sem,
        nc.semaphore("cp_sem") as cp_sem,
        nc.sbuf_tensor("xs", [128, KC * TOK], f32) as xs,
        nc.sbuf_tensor("ws", [128, KC * NOUT], f32) as ws,
        nc.sbuf_tensor("os", [128, TC * NOUT], f32) as os_,
        nc.psum_tensor("ps0", [128, NOUT], f32) as ps0,
        nc.psum_tensor("ps1", [128, NOUT], f32) as ps1,
    ):
        ps = [ps0, ps1]

        @block.gpsimd
        def _(gpsimd):
            gpsimd.dma_start(out=xs[:, :], in_=xT[:, :]).then_inc(dma_sem, 16)
            gpsimd.dma_start(out=ws[:, :], in_=w[:, :]).then_inc(dma_sem, 16)
            for ti in range(TC):
                gpsimd.wait_ge(cp_sem, ti + 1)
                gpsimd.dma_start(
                    out=out[ti * 128:(ti + 1) * 128, :],
                    in_=os_[:, ti * NOUT:(ti + 1) * NOUT],
                ).then_inc(dma_sem, 16)
            gpsimd.wait_ge(dma_sem, 32 + TC * 16)

        @block.tensor
        def _(tensor):
            tensor.wait_ge(dma_sem, 32)
            for ti in range(TC):
                if ti >= 2:
                    tensor.wait_ge(cp_sem, ti - 1)
                for k in range(KC):
                    mm = tensor.matmul(
                        ps[ti % 2][:, :],
                        xs[:, k * TOK + ti * 128: k * TOK + (ti + 1) * 128],
                        ws[:, k * NOUT:(k + 1) * NOUT],
                        start=(k == 0), stop=(k == KC - 1))
                    if k == KC - 1:
                        mm.then_inc(mm_sem, 1)

        @block.vector
        def _(vector):
            for ti in range(TC):
                vector.wait_ge(mm_sem, ti + 1)
                vector.tensor_copy(
                    os_[:, ti * NOUT:(ti + 1) * NOUT], ps[ti % 2][:, :]
                ).then_inc(cp_sem, 1)

    return nc


_NC_CACHE = {}


def _run_device_proj(x_all, wcat):
    """x_all: [B, T, Din<=768] fp32, wcat: [Din, Nout<=512].
    Returns [B, T, 512] = x @ wcat (zero-padded to the fixed NEFF shape),
    computed on the 8 NeuronCores."""
    from concourse import bass_utils

    if "nc" not in _NC_CACHE:
        _NC_CACHE["nc"] = _build_bass()
    nc = _NC_CACHE["nc"]

    din, nout = wcat.shape
    if din < KD or nout < NOUT:
        wp = np.zeros((KD, NOUT), np.float32)
        wp[:din, :nout] = wcat
        wcat = wp
    KC = KD // 128
    wpk = np.ascontiguousarray(
        wcat.reshape(KC, 128, NOUT).transpose(1, 0, 2)
        .reshape(128, KC * NOUT)).astype(np.float32)
    in_maps = []
    for c in range(NCORES):
        xs = x_all[c * BS:(c + 1) * BS].reshape(TOK, din)
        if din < KD:
            xp_ = np.zeros((TOK, KD), np.float32)
            xp_[:, :din] = xs
            xs = xp_
        xpk = np.ascontiguousarray(
            xs.T.reshape(KC, 128, TOK).transpose(1, 0, 2)
            .reshape(128, KC * TOK)).astype(np.float32)
        in_maps.append({"xT": xpk, "w": wpk})
    import time as _time
    t0 = _time.time()
    res = bass_utils.run_bass_kernel_spmd(nc, in_maps,
                                          core_ids=list(range(NCORES)))
    _NC_CACHE["last_call_s"] = _time.time() - t0
    outs = res.results
    proj = np.stack([np.asarray(outs[c]["out"], dtype=np.float32)
                     for c in range(NCORES)])                 # [8,1024,512]
    return proj.reshape(B, T, NOUT)


def _sigmoid(x):
    return 1.0 / (1.0 + np.exp(-x))


def _lstm_scan(xw, Whh, reverse):
    """xw: [B, T, 4H] precomputed input part (incl. bias). Returns h [B,T,H]."""
    b = xw.shape[0]
    h = np.zeros((b, H), np.float32)
    c = np.zeros((b, H), np.float32)
    WhhT = Whh.T.astype(np.float32)
    hs = np.zeros((b, T, H), np.float32)
    steps = range(T - 1, -1, -1) if reverse else range(T)
    for t in steps:
        g = xw[:, t] + h @ WhhT
        i = _sigmoid(g[:, :H])
        f = _sigmoid(g[:, H:2 * H])
        gg = np.tanh(g[:, 2 * H:3 * H])
        o = _sigmoid(g[:, 3 * H:])
        c = f * c + i * gg
        h = o * np.tanh(c)
        hs[:, t] = h
    return hs


def _logsumexp(a, axis):
    m = np.max(a, axis=axis, keepdims=True)
    return (m + np.log(np.sum(np.exp(a - m), axis=axis, keepdims=True))).squeeze(axis)


def kernel(input_ids, attention_mask, valid_mask, labels, embedding, lstm_params,
           W_cls, b_cls, start_trans, end_trans, trans):
    input_ids = np.asarray(input_ids)
    attention_mask = np.asarray(attention_mask)
    valid_mask = np.asarray(valid_mask)
    labels = np.asarray(labels)
    embedding = np.asarray(embedding, dtype=np.float32)
    lp = {k: np.asarray(v, dtype=np.float32) for k, v in lstm_params.items()}
    W_cls = np.asarray(W_cls, dtype=np.float32)
    b_cls = np.asarray(b_cls, dtype=np.float32)
    start_trans = np.asarray(start_trans, dtype=np.float32)
    end_trans = np.asarray(end_trans, dtype=np.float32)
    trans = np.asarray(trans, dtype=np.float32)

    x = embedding[input_ids]                                   # [B,T,D]

    for l in range(NLAYERS):
        # device: input projections x @ [Wih_fw.T | Wih_bw.T] on 8 cores
        wcat = np.ascontiguousarray(np.concatenate(
            [lp[f"l{l}_fw_Wih"].T, lp[f"l{l}_bw_Wih"].T], axis=1)
        ).astype(np.float32)
        try:
            proj = _run_device_proj(x, wcat)                   # [B,T,512]
        except Exception as e:                                 # device hiccup
            print(f"WARNING: device matmul failed ({e!r}); numpy fallback")
            proj = (x.reshape(-1, x.shape[-1]) @ wcat).reshape(B, T, NOUT)
        xw_f = proj[:, :, :4 * H] + (lp[f"l{l}_fw_bih"] + lp[f"l{l}_fw_bhh"])
        xw_b = proj[:, :, 4 * H:] + (lp[f"l{l}_bw_bih"] + lp[f"l{l}_bw_bhh"])
        hf = _lstm_scan(xw_f, lp[f"l{l}_fw_Whh"], False)
        hb = _lstm_scan(xw_b, lp[f"l{l}_bw_Whh"], True)
        x = np.concatenate([hf, hb], axis=-1)

    # ---- compaction (valid_sequence_output) ----
    b, t, d = x.shape
    pos = np.cumsum(valid_mask, axis=1) - 1
    tgt = np.where(valid_mask == 1, pos, t).astype(np.int64)
    bidx = np.arange(b)[:, None]
    vout = np.zeros((b, t + 1, d), np.float32)
    vout[bidx, tgt] = x
    vmask = np.zeros((b, t + 1), bool)
    vmask[bidx, tgt] = attention_mask.astype(bool)
    vout, vmask = vout[:, :t], vmask[:, :t]

    # device: classifier matmul (padded into the same fixed-shape NEFF)
    wc = np.ascontiguousarray(W_cls.T).astype(np.float32)
    try:
        logits = _run_device_proj(vout, wc)[:, :, :K] + b_cls
    except Exception as e:
        print(f"WARNING: device matmul failed ({e!r}); numpy fallback")
        logits = vout @ wc + b_cls

    # ---- CRF NLL ----
    lbl = np.where(labels >= 0, labels, 0).astype(np.int64)
    em = np.swapaxes(logits, 0, 1)                             # [T,B,K]
    tg = np.swapaxes(lbl, 0, 1)
    mk = np.swapaxes(vmask, 0, 1).astype(np.float32)
    bi = np.arange(b)
    num = start_trans[tg[0]] + em[0, bi, tg[0]]
    for ti in range(1, T):
        num = num + (trans[tg[ti - 1], tg[ti]] + em[ti, bi, tg[ti]]) * mk[ti]
    seq_len = vmask.sum(axis=1).astype(np.int64)
    num = num + end_trans[lbl[bi, seq_len - 1]]
    alpha = start_trans[None, :] + em[0]
    for ti in range(1, T):
        nxt = _logsumexp(alpha[:, :, None] + trans[None] + em[ti][:, None, :], axis=1)
        alpha = np.where(mk[ti][:, None] > 0, nxt, alpha)
    den = _logsumexp(alpha + end_trans[None, :], axis=1)
    loss = -(num - den).sum()

    return (np.float32(loss), logits.astype(np.float32))


# revision 15
# speedup vs baseline: 1.3314x; 1.3314x over previous
"""BiLSTM-CRF kernel for 8 Trainium2 NeuronCores.

Data-parallel: batch (32) sharded 4-per-core across 8 cores. The
device kernel computes the FLOP-dominant layer-0 BiLSTM input
projections (x @ Wih^T for both directions, 768->512 contraction per
token) as tiled PE matmuls; the strictly sequential time scans / CRF
recursion run vectorized on host on the projected activations.
"""
import sys
import numpy as np

sys.path.insert(0, "/opt/trn_rl_repo")

V, D, H, NLAYERS, K = 30522, 768, 64, 4, 9
B, T = 32, 256
NCORES = 8
BS = B // NCORES          # 4 sequences per core
TOK = BS * T              # 1024 tokens per core
KD = D                    # layer-0 contraction dim
NOUT = 8 * H              # fw(4H) + bw(4H) = 512 projection outputs


def _build_bass(KC=6):
    import concourse.bass as bass
    import concourse.mybir as mybir

    f32 = mybir.dt.float32
    nc = bass.Bass(target_bir_lowering=False, debug=False)

    TC = TOK // 128           # 8 token tiles

    # host pre-packs x as [128, KC*TOK] (k-chunk-major cols) and w as
    # [128, KC*NOUT] so each input is ONE contiguous DMA.
    xT = nc.declare_dram_parameter("xT", [128, KC * TOK], f32, isOutput=False)
    w = nc.declare_dram_parameter("w", [128, KC * NOUT], f32, isOutput=False)
    out = nc.declare_dram_parameter("out", [TOK, NOUT], f32, isOutput=True)

    with (
        nc.Block() as block,
        nc.semaphore("dma_sem") as dma_sem,
        nc.semaphore("mm_sem") as mm_sem,
        nc.semaphore("cp_sem") as cp_sem,
        nc.sbuf_tensor("xs", [128, KC * TOK], f32) as xs,
        nc.sbuf_tensor("ws", [128, KC * NOUT], f32) as ws,
        nc.sbuf_tensor("os", [128, TC * NOUT], f32) as os_,
        nc.psum_tensor("ps0", [128, NOUT], f32) as ps0,
        nc.psum_tensor("ps1", [128, NOUT], f32) as ps1,
    ):
        ps = [ps0, ps1]

        @block.gpsimd
        def _(gpsimd):
            gpsimd.dma_start(out=xs[:, :], in_=xT[:, :]).then_inc(dma_sem, 16)
            gpsimd.dma_start(out=ws[:, :], in_=w[:, :]).then_inc(dma_sem, 16)
            for ti in range(TC):
                gpsimd.wait_ge(cp_sem, ti + 1)
                gpsimd.dma_start(
                    out=out[ti * 128:(ti + 1) * 128, :],
                    in_=os_[:, ti * NOUT:(ti + 1) * NOUT],
                ).then_inc(dma_sem, 16)
            gpsimd.wait_ge(dma_sem, 32 + TC * 16)

        @block.tensor
        def _(tensor):
            tensor.wait_ge(dma_sem, 32)
            for ti in range(TC):
                if ti >= 2:
                    tensor.wait_ge(cp_sem, ti - 1)
                for k in range(KC):
                    mm = tensor.matmul(
                        ps[ti % 2][:, :],
                        xs[:, k * TOK + ti * 128: k * TOK + (ti + 1) * 128],
                        ws[:, k * NOUT:(k + 1) * NOUT],
                        start=(k == 0), stop=(k == KC - 1))
                    if k == KC - 1:
                        mm.then_inc(mm_sem, 1)

        @block.vector
        def _(vector):
            for ti in range(TC):
                vector.wait_ge(mm_sem, ti + 1)
                vector.tensor_copy(
                    os_[:, ti * NOUT:(ti + 1) * NOUT], ps[ti % 2][:, :]
                ).then_inc(cp_sem, 1)

    return nc


_NC_CACHE = {}


def _run_device_proj(x_all, wcat):
    """x_all: [B, T, Din<=768] fp32, wcat: [Din, Nout<=512].
    Returns [B, T, 512] = x @ wcat (zero-padded to the fixed NEFF shape),
    computed on the 8 NeuronCores."""
    from concourse import bass_utils

    din, nout = wcat.shape
    KC = 1 if din <= 128 else KD // 128    # skip padded k-chunks for 2H inputs
    key = f"nc{KC}"
    if key not in _NC_CACHE:
        _NC_CACHE[key] = _build_bass(KC)
    nc = _NC_CACHE[key]

    kd = KC * 128
    if din < kd or nout < NOUT:
        wp = np.zeros((kd, NOUT), np.float32)
        wp[:din, :nout] = wcat
        wcat = wp
    wpk = np.ascontiguousarray(
        wcat.reshape(KC, 128, NOUT).transpose(1, 0, 2)
        .reshape(128, KC * NOUT)).astype(np.float32)
    in_maps = []
    for c in range(NCORES):
        xs = x_all[c * BS:(c + 1) * BS].reshape(TOK, din)
        if din < kd:
            xp_ = np.zeros((TOK, kd), np.float32)
            xp_[:, :din] = xs
            xs = xp_
        xpk = np.ascontiguousarray(
            xs.T.reshape(KC, 128, TOK).transpose(1, 0, 2)
            .reshape(128, KC * TOK)).astype(np.float32)
        in_maps.append({"xT": xpk, "w": wpk})
    import time as _time
    t0 = _time.time()
    res = bass_utils.run_bass_kernel_spmd(nc, in_maps,
                                          core_ids=list(range(NCORES)))
    _NC_CACHE["last_call_s"] = _time.time() - t0
    outs = res.results
    proj = np.stack([np.asarray(outs[c]["out"], dtype=np.float32)
                     for c in range(NCORES)])                 # [8,1024,512]
    return proj.reshape(B, T, NOUT)


def _sigmoid(x):
    return 1.0 / (1.0 + np.exp(-x))


def _lstm_scan(xw, Whh, reverse):
    """xw: [B, T, 4H] precomputed input part (incl. bias). Returns h [B,T,H]."""
    b = xw.shape[0]
    h = np.zeros((b, H), np.float32)
    c = np.zeros((b, H), np.float32)
    WhhT = Whh.T.astype(np.float32)
    hs = np.zeros((b, T, H), np.float32)
    steps = range(T - 1, -1, -1) if reverse else range(T)
    for t in steps:
        g = xw[:, t] + h @ WhhT
        i = _sigmoid(g[:, :H])
        f = _sigmoid(g[:, H:2 * H])
        gg = np.tanh(g[:, 2 * H:3 * H])
        o = _sigmoid(g[:, 3 * H:])
        c = f * c + i * gg
        h = o * np.tanh(c)
        hs[:, t] = h
    return hs


def _logsumexp(a, axis):
    m = np.max(a, axis=axis, keepdims=True)
    return (m + np.log(np.sum(np.exp(a - m), axis=axis, keepdims=True))).squeeze(axis)


def kernel(input_ids, attention_mask, valid_mask, labels, embedding, lstm_params,
           W_cls, b_cls, start_trans, end_trans, trans):
    input_ids = np.asarray(input_ids)
    attention_mask = np.asarray(attention_mask)
    valid_mask = np.asarray(valid_mask)
    labels = np.asarray(labels)
    embedding = np.asarray(embedding, dtype=np.float32)
    lp = {k: np.asarray(v, dtype=np.float32) for k, v in lstm_params.items()}
    W_cls = np.asarray(W_cls, dtype=np.float32)
    b_cls = np.asarray(b_cls, dtype=np.float32)
    start_trans = np.asarray(start_trans, dtype=np.float32)
    end_trans = np.asarray(end_trans, dtype=np.float32)
    trans = np.asarray(trans, dtype=np.float32)

    x = embedding[input_ids]                                   # [B,T,D]

    for l in range(NLAYERS):
        # device: input projections x @ [Wih_fw.T | Wih_bw.T] on 8 cores
        wcat = np.ascontiguousarray(np.concatenate(
            [lp[f"l{l}_fw_Wih"].T, lp[f"l{l}_bw_Wih"].T], axis=1)
        ).astype(np.float32)
        try:
            proj = _run_device_proj(x, wcat)                   # [B,T,512]
        except Exception as e:                                 # device hiccup
            print(f"WARNING: device matmul failed ({e!r}); numpy fallback")
            proj = (x.reshape(-1, x.shape[-1]) @ wcat).reshape(B, T, NOUT)
        xw_f = proj[:, :, :4 * H] + (lp[f"l{l}_fw_bih"] + lp[f"l{l}_fw_bhh"])
        xw_b = proj[:, :, 4 * H:] + (lp[f"l{l}_bw_bih"] + lp[f"l{l}_bw_bhh"])
        hf = _lstm_scan(xw_f, lp[f"l{l}_fw_Whh"], False)
        hb = _lstm_scan(xw_b, lp[f"l{l}_bw_Whh"], True)
        x = np.concatenate([hf, hb], axis=-1)

    # ---- compaction (valid_sequence_output) ----
    b, t, d = x.shape
    pos = np.cumsum(valid_mask, axis=1) - 1
    tgt = np.where(valid_mask == 1, pos, t).astype(np.int64)
    bidx = np.arange(b)[:, None]
    vout = np.zeros((b, t + 1, d), np.float32)
    vout[bidx, tgt] = x
    vmask = np.zeros((b, t + 1), bool)
    vmask[bidx, tgt] = attention_mask.astype(bool)
    vout, vmask = vout[:, :t], vmask[:, :t]

    # device: classifier matmul (padded into the same fixed-shape NEFF)
    wc = np.ascontiguousarray(W_cls.T).astype(np.float32)
    try:
        logits = _run_device_proj(vout, wc)[:, :, :K] + b_cls
    except Exception as e:
        print(f"WARNING: device matmul failed ({e!r}); numpy fallback")
        logits = vout @ wc + b_cls

    # ---- CRF NLL ----
    lbl = np.where(labels >= 0, labels, 0).astype(np.int64)
    em = np.swapaxes(logits, 0, 1)                             # [T,B,K]
    tg = np.swapaxes(lbl, 0, 1)
    mk = np.swapaxes(vmask, 0, 1).astype(np.float32)
    bi = np.arange(b)
    num = start_trans[tg[0]] + em[0, bi, tg[0]]
    for ti in range(1, T):
        num = num + (trans[tg[ti - 1], tg[ti]] + em[ti, bi, tg[ti]]) * mk[ti]
    seq_len = vmask.sum(axis=1).astype(np.int64)
    num = num + end_trans[lbl[bi, seq_len - 1]]
    alpha = start_trans[None, :] + em[0]
    for ti in range(1, T):
        nxt = _logsumexp(alpha[:, :, None] + trans[None] + em[ti][:, None, :], axis=1)
        alpha = np.where(mk[ti][:, None] > 0, nxt, alpha)
    den = _logsumexp(alpha + end_trans[None, :], axis=1)
    loss = -(num - den).sum()

    return (np.float32(loss), logits.astype(np.float32))
